# revision 19
# baseline (speedup 1.0000x reference)
"""Trainium2 kernel for nn_AEDecoder: out = LeakyReLU(X @ W_sparse + bias).

The sparse edge list (400k edges over a [1639, 17000] weight matrix, 1.4%
dense) is converted on the host to a dense weight matrix — the layout the
TensorEngine consumes — with the bias folded in as an extra ones-row of X.
Each of the 8 NeuronCores gets a 2125-gene column shard of W (data-parallel
over output genes, X replicated), runs a tiled matmul with f32 PSUM
accumulation and a fused LeakyReLU epilogue, and the host concatenates the
per-core outputs.

Device schedule (per core), v2 — rebuilt around the measured trace of the
65.5us baseline (PE-stream 98% dense; the remaining time was preamble/DMA
start ~5us, drain tail 3.7us, and a fixed ~7.9us runtime postamble):
- The warmup scratch is memset BEFORE the tile context (it executes in the
  engine-preamble region), so the dep-free warmup matmuls issue the moment
  the PE clears the preamble barrier (~5.6us) instead of waiting on an
  in-body memset chain (~7.6us). 14 N=128 warmups bridge to the first
  chunk's arrival and start the HAM clock ramp early.
- First-chunk DMAs are front-of-queue on both HWDGE rings (x0 on sync,
  wa0 split in two 512-col halves on scalar) so the k=0 matmuls can start
  ~3us earlier than the old 30-warmup schedule allowed.
- Two of the 13 K-chunks (k=4,5; not the bias chunk) are computed as a
  single fp8e4 DoubleRow matmul per (m,n) tile: both 128-row chunk
  products are packed into one 216ns PE pass (2x MAC rate), saving ~3.5us
  of PE time. Host pre-scales W by 8 and X by 1/8 to keep e4m3 operands in
  the normal range; measured end-to-end rel err 1.47e-2 vs the 2e-2 gate
  (bf16-only is 2.3e-3; each extra fp8 chunk adds ~sqrt(1/13)*3.7e-2).
- Pass 1 computes genes 0..1023 k-outer (chunk k consumed as it lands);
  pass 2 genes 1024..2047 for batch rows 0..255 k-outer; then the
  remaining work is ordered so the kernel END is tiny: full-width sweeps
  for (n2,m2) and (n3,m2), the transposed 77-gene tail (drained
  mid-stream), sweep (n2,m3), and finally sweep (n3,m3) split into
  256/128/128-wide pieces so only a [128,128] LeakyReLU + 32KB write +
  receipt (~1.2us) follow the last matmul.
- Mid-stream LeakyReLU drains alternate between the ACT engine (fused
  Lrelu) and the DVE (0.01x + 0.99*relu(x)) so banks release at twice the
  single-engine rate; outputs are written bf16 and upcast on the host.
"""

import os
import sys

import numpy as np

for _p in ("/opt/trn_rl_repo", "/root/.axon_site/_ro/trn_rl_repo"):
    if _p not in sys.path:
        sys.path.append(_p)

import ml_dtypes

B, IN_F, OUT_F = 512, 1639, 17000
NCORES = 8
SHARD = OUT_F // NCORES      # 2125 output genes per core
K_PAD = 1640                 # 1639 TF rows + 1 bias row (last chunk K=104)
KC = 13                      # contraction chunks (12 x 128 + 1 x 104)
NEG_SLOPE = 0.01
NTILE = 512                  # PSUM bank width in f32
NMAIN = (SHARD // NTILE) * NTILE   # 2048 genes in batch-major layout
NTAIL = SHARD - NMAIN              # 77 genes in gene-major (transposed) layout
MC = B // 128                # 4 batch chunks
WARMUP_MM = 26               # cheap N=128 matmuls to ramp the HAM clock gate
FP8_CHUNKS = () if os.environ.get("AED_NO_FP8") else (4, 5)
FP8_SCALE = 8.0              # host pre-scale: W*8, X/8 keep e4m3 in normal range
E3_CHUNKS = () if os.environ.get("AED_NO_FP8") else (1, 2, 3, 6, 7)
E3_SCALE = 32.0              # wa shipped as e3m4 (W*32), upcast on DVE

_cache: dict = {}


def _build_nc():
    import concourse.tile as tile
    from concourse import bacc, mybir
    nc = bacc.Bacc(
        "TRN2",
        target_bir_lowering=False,
        debug=False,
        num_devices=NCORES,
    )
    NB2 = SHARD - 2 * NTILE
    xT = nc.dram_tensor("xT", [K_PAD, B], mybir.dt.bfloat16, kind="ExternalInput").ap()
    # W arrives pre-split into the pass-1 and pass-2 column blocks so every
    # 128-row chunk DMA reads a fully contiguous DRAM range.
    wA = nc.dram_tensor(
        "wa", [K_PAD, 2 * NTILE], mybir.dt.bfloat16, kind="ExternalInput"
    ).ap()
    wB = nc.dram_tensor(
        "wb", [K_PAD, NB2], mybir.dt.bfloat16, kind="ExternalInput"
    ).ap()
    if FP8_CHUNKS:
        x8d = nc.dram_tensor(
            "x8", [128, 2, B], mybir.dt.float8e4, kind="ExternalInput"
        ).ap()
        w8lod = nc.dram_tensor(
            "w8lo", [128, 2, 2 * NTILE], mybir.dt.float8e4, kind="ExternalInput"
        ).ap()
        w8hid = nc.dram_tensor(
            "w8hi", [128, 2, 2 * NTILE], mybir.dt.float8e4, kind="ExternalInput"
        ).ap()
        wt45d = nc.dram_tensor(
            "wt45", [2 * 128, NTAIL], mybir.dt.bfloat16, kind="ExternalInput"
        ).ap()
    if E3_CHUNKS:
        wa8d = nc.dram_tensor(
            "wa8", [len(E3_CHUNKS) * 128, 2 * NTILE], mybir.dt.float8e3,
            kind="ExternalInput",
        ).ap()
    out = nc.dram_tensor("out", [B, NMAIN], mybir.dt.bfloat16, kind="ExternalOutput").ap()
    out2 = nc.dram_tensor(
        "out2", [B, NTAIL], mybir.dt.bfloat16, kind="ExternalOutput"
    ).ap()

    bf16 = mybir.dt.bfloat16
    f32 = mybir.dt.float32
    fp8 = mybir.dt.float8e4
    Lrelu = mybir.ActivationFunctionType.Lrelu
    DR = mybir.MatmulPerfMode.DoubleRow

    # Warmup scratch initialized before the tile context: the memset lands in
    # the engine-preamble region, so the warmup matmuls have no in-body deps
    # and the PE starts the HAM ramp right after the preamble barrier.
    scr_h = nc.alloc_sbuf_tensor("warm_scr", [128, 128], bf16)
    nc.vector.memset(scr_h.ap(), 0.0)
    scr = scr_h.ap()

    with tile.TileContext(nc) as tc:
        with (
            tc.tile_pool(name="xp", bufs=1) as xp,
            tc.tile_pool(name="wp", bufs=1) as wp,
            tc.tile_pool(name="pp", bufs=8, space="PSUM") as pp,
            tc.tile_pool(name="op", bufs=6) as op,
        ):
            scr_ps = pp.tile([128, 128], f32, tag="psum", name="scr_ps")
            for _ in range(WARMUP_MM):
                nc.tensor.matmul(scr_ps[:], lhsT=scr, rhs=scr, start=True, stop=True)

            # --- input stream, deadline-ordered per DMA ring ---
            # Chunk 0 is split into four half-tiles spread over all three
            # rings so the very first k=0 matmuls (n0 x m0/m1) wait on just
            # 65KB + 131KB landing in parallel instead of 393KB serialized.
            bf_chunks = [k for k in range(KC) if k not in FP8_CHUNKS]
            xts, was, wbs = [None], {}, {}
            x0a = xp.tile([128, B // 2], bf16, tag="x0a", name="x0a")
            x0b = xp.tile([128, B // 2], bf16, tag="x0b", name="x0b")
            wa0n0 = wp.tile([128, NTILE], bf16, tag="wa0n0", name="wa0n0")
            wa0n1 = wp.tile([128, NTILE], bf16, tag="wa0n1", name="wa0n1")
            for k in range(1, KC):
                kr = min(128, K_PAD - k * 128)
                xts.append(xp.tile([kr, B], bf16, tag=f"x{k}", name=f"x_{k}"))
            wa3s = {}
            for k in bf_chunks:
                kr = min(128, K_PAD - k * 128)
                if k != 0:
                    was[k] = wp.tile(
                        [kr, 2 * NTILE], bf16, tag=f"wa{k}", name=f"wa_{k}"
                    )
                    if k in E3_CHUNKS:
                        wa3s[k] = wp.tile(
                            [kr, 2 * NTILE], mybir.dt.float8e3,
                            tag=f"wa3{k}", name=f"wa3_{k}",
                        )
                wbs[k] = wp.tile([kr, NB2], bf16, tag=f"wb{k}", name=f"wb_{k}")
            if FP8_CHUNKS:
                x8 = xp.tile([128, 2, B], fp8, tag="x8pair", name="x8pair")
                w8lo = wp.tile([128, 2, 2 * NTILE], fp8, tag="w8lo", name="w8lo")
                w8hi = wp.tile([128, 2, 2 * NTILE], fp8, tag="w8hi", name="w8hi")
                wt4 = wp.tile([128, NTAIL], bf16, tag="wt4", name="wt4")
                wt5 = wp.tile([128, NTAIL], bf16, tag="wt5", name="wt5")

            # The early input phase is ring-bandwidth + per-item-receipt
            # bound, so items are assigned in global deadline order to the
            # ring with the earliest estimated finish (sync/scalar HWDGE
            # ~120GB/s, gpsimd SWDGE ~100GB/s, ~0.3us receipt per item).
            # x4/x5 (only read by the late 77-gene tail) and all wb chunks
            # (pass 2 onward) go to the back.
            front = [
                (x0a, xT[0:128, 0 : B // 2]),
                (wa0n0, wA[0:128, 0:NTILE]),
                (x0b, xT[0:128, B // 2 : B]),
                (wa0n1, wA[0:128, NTILE:]),
            ]
            for k in range(1, KC):
                kr = min(128, K_PAD - k * 128)
                if k in FP8_CHUNKS:
                    if k == FP8_CHUNKS[0]:
                        front.append((x8, x8d))
                        front.append((w8lo, w8lod))
                    continue
                front.append((xts[k], xT[k * 128 : k * 128 + kr, :]))
                if k in E3_CHUNKS:
                    j = E3_CHUNKS.index(k)
                    front.append((wa3s[k], wa8d[j * 128 : (j + 1) * 128, :]))
                else:
                    front.append((was[k], wA[k * 128 : k * 128 + kr, :]))
            back = []
            for k in bf_chunks:
                kr = min(128, K_PAD - k * 128)
                back.append((wbs[k], wB[k * 128 : k * 128 + kr, :]))
            if FP8_CHUNKS:
                back.append((w8hi, w8hid))
                for k in FP8_CHUNKS:
                    back.append((xts[k], xT[k * 128 : (k + 1) * 128, :]))
                back.append((wt4, wt45d[0:128, :]))
                back.append((wt5, wt45d[128:256, :]))
            rings = [(nc.sync, 120.0), (nc.scalar, 120.0), (nc.gpsimd, 100.0)]
            finish = [0.0, 0.0, 0.3]
            arr = {}
            for tile_t, src_ap in front + back:
                nbytes = mybir.dt.size(tile_t.dtype)
                for d in tile_t.shape:
                    nbytes *= d
                i = min(range(3), key=lambda j: finish[j])
                finish[i] += nbytes / (rings[i][1] * 1e3) + 0.3
                arr[id(tile_t)] = finish[i]
                rings[i][0].dma_start(tile_t[:], src_ap)

            # e3m4 wa chunks: upcast to bf16 on the (otherwise idle) DVE
            for k in E3_CHUNKS:
                nc.vector.tensor_scalar_mul(was[k][:], wa3s[k][:], 1.0 / E3_SCALE)

            # Per-chunk predicted-arrival times drive the k-outer consumption
            # order (PSUM accumulation commutes), so an early-arriving chunk
            # absorbs another chunk's lateness instead of stalling the PE.
            def ready_p1(k):
                if FP8_CHUNKS and k == FP8_CHUNKS[0]:
                    return max(arr[id(x8)], arr[id(w8lo)])
                return max(arr[id(xts[k])], arr[id(was[k])])

            def ready_p2(k):
                if FP8_CHUNKS and k == FP8_CHUNKS[0]:
                    return max(arr[id(x8)], arr[id(w8hi)])
                return arr[id(wbs[k])]

            live = [k for k in range(1, KC) if k not in FP8_CHUNKS]
            if FP8_CHUNKS:
                live.append(FP8_CHUNKS[0])
                live.sort()
            p1_ks = [0] + live
            p2_ks = [0] + sorted(live, key=ready_p2)

            def x_slice(k, m):
                if k == 0:
                    t = x0a if m < 2 else x0b
                    return t[:, (m % 2) * 128 : (m % 2 + 1) * 128]
                return xts[k][:, m * 128 : (m + 1) * 128]

            def w_slice(k, n):
                if n < 2:
                    if k == 0:
                        return (wa0n0 if n == 0 else wa0n1)[:, :]
                    return was[k][:, n * NTILE : (n + 1) * NTILE]
                return wbs[k][:, (n - 2) * NTILE : (n - 1) * NTILE]

            def w8_slice(n, c0, c1):
                t = w8lo if n < 2 else w8hi
                base = (n % 2) * NTILE
                return t[:, :, base + c0 : base + c1]

            def emit_k(ps, m, n, k, c0=0, c1=NTILE, first=None, last=None):
                """Emit the chunk-k matmul(s) for psum ps covering gene cols
                [c0:c1) of n-tile n and batch chunk m. At k=FP8_CHUNKS[0] a
                single DoubleRow matmul covers both fp8 chunks."""
                if first is None:
                    first = k == 0
                if last is None:
                    last = k == KC - 1
                if FP8_CHUNKS and k == FP8_CHUNKS[0]:
                    nc.tensor.matmul(
                        ps[:],
                        lhsT=x8[:, :, m * 128 : (m + 1) * 128],
                        rhs=w8_slice(n, c0, c1),
                        start=first,
                        stop=last,
                        perf_mode=DR,
                    )
                    return
                if FP8_CHUNKS and k in FP8_CHUNKS:
                    return
                nc.tensor.matmul(
                    ps[:],
                    lhsT=x_slice(k, m),
                    rhs=w_slice(k, n)[:, c0:c1],
                    start=first,
                    stop=last,
                )

            drain_i = [0]

            def out_ring(nbytes):
                """Pick the least-loaded DMA ring (same greedy state as the
                input stream) for an output write."""
                i = min(range(3), key=lambda j: finish[j])
                finish[i] += nbytes / (rings[i][1] * 1e3) + 0.3
                return rings[i][0]

            def drain(ps, n, m, c0=0, c1=NTILE, eng_override=None, act_only=False):
                """LeakyReLU PSUM->SBUF (alternating ACT/DVE) + out DMA."""
                i = drain_i[0]
                drain_i[0] += 1
                w = c1 - c0
                ot = op.tile([128, w], bf16, tag="o", name=f"o_{n}_{m}_{c0}")
                if act_only or i % 2 == 0:
                    nc.scalar.activation(ot[:], ps[:], Lrelu, alpha=NEG_SLOPE)
                else:
                    # lrelu(x) = 0.01x + 0.99*relu(x); PSUM may only be read
                    # once per DVE instruction, so stage relu in SBUF.
                    rt = op.tile([128, w], bf16, tag="r", name=f"r_{n}_{m}_{c0}")
                    nc.vector.tensor_scalar(
                        rt[:], ps[:], 0.0, 1.0 - NEG_SLOPE,
                        mybir.AluOpType.max, mybir.AluOpType.mult,
                    )
                    nc.vector.scalar_tensor_tensor(
                        ot[:], ps[:], NEG_SLOPE, rt[:],
                        mybir.AluOpType.mult, mybir.AluOpType.add,
                    )
                eng = eng_override if eng_override is not None else out_ring(w * 256)
                eng.dma_start(
                    out[m * 128 : (m + 1) * 128, n * NTILE + c0 : n * NTILE + c1],
                    ot[:],
                )

            def k_outer_block(ns, ms, ks):
                """k-outer accumulation for the (n, m) tile set, consuming
                chunks in predicted-arrival order. Returns the psum tiles
                keyed by (n, m)."""
                pts = {
                    (n, m): pp.tile([128, NTILE], f32, tag="psum", name=f"ps_{n}_{m}")
                    for n in ns
                    for m in ms
                }
                for ki, k in enumerate(ks):
                    if k == 0:
                        # m0/m1 read x0a (sync, lands first); m2/m3 read x0b
                        order = [
                            (n, m)
                            for mg in ((0, 1), (2, 3))
                            for n in ns
                            for m in mg
                            if m in ms
                        ]
                    else:
                        order = [(n, m) for n in ns for m in ms]
                    for n, m in order:
                        emit_k(
                            pts[(n, m)], m, n, k,
                            first=(ki == 0), last=(ki == len(ks) - 1),
                        )
                return pts

            def nm_sweep(n, m, eng, c0=0, c1=NTILE):
                """Single-bank k-contiguous sweep, ACT drain, HWDGE write."""
                w = c1 - c0
                ps = pp.tile([128, w], f32, tag="psum", name=f"ps_{n}_{m}_{c0}")
                ks = [0] + live
                for ki, k in enumerate(ks):
                    emit_k(
                        ps, m, n, k, c0, c1,
                        first=(ki == 0), last=(ki == len(ks) - 1),
                    )
                ot = op.tile([128, w], bf16, tag="o", name=f"os_{n}_{m}_{c0}")
                nc.scalar.activation(ot[:], ps[:], Lrelu, alpha=NEG_SLOPE)
                eng.dma_start(
                    out[m * 128 : (m + 1) * 128, n * NTILE + c0 : n * NTILE + c1],
                    ot[:],
                )

            # Pass 1: genes 0..1023, all batch rows, k-outer (DMA-paced).
            p1 = k_outer_block((0, 1), range(MC), p1_ks)
            for n in (0, 1):
                for m in range(MC):
                    drain(p1[(n, m)], n, m)
            # Pass 2: genes 1024..2047, batch rows 0..255, k-outer.
            p2 = k_outer_block((2, 3), (0, 1), p2_ks)
            for n in (2, 3):
                for m in (0, 1):
                    drain(p2[(n, m)], n, m)

            # Batch rows 256..511: single-bank k-contiguous sweeps staggered
            # so each drain+DMA overlaps the next sweep's matmuls; the
            # transposed 77-gene tail runs mid-sequence so its drain is off
            # the critical path, and the final sweep is split into shrinking
            # pieces so only a [128,128] drain + 32KB write end the kernel.
            nm_sweep(2, 2, nc.sync)
            nm_sweep(3, 2, nc.scalar)
            # 77-gene tail in normal orientation: 13 k-contig matmuls of
            # width 77 per batch chunk (32ns each at full clock vs 216ns for
            # the transposed 512-wide form), drained mid-stream on gpsimd.
            for m in range(MC):
                tps = pp.tile([128, NTAIL], f32, tag="psum", name=f"tail_{m}")
                tks = [k for k in range(KC)]
                for ki, k in enumerate(tks):
                    if FP8_CHUNKS and k in FP8_CHUNKS:
                        rhs = (wt4 if k == FP8_CHUNKS[0] else wt5)[:]
                    else:
                        rhs = wbs[k][:, NMAIN - 2 * NTILE : NB2]
                    nc.tensor.matmul(
                        tps[:],
                        lhsT=x_slice(k, m),
                        rhs=rhs,
                        start=(ki == 0),
                        stop=(ki == len(tks) - 1),
                    )
                tot = op.tile([128, NTAIL], bf16, tag="o", name=f"otail_{m}")
                nc.scalar.activation(tot[:], tps[:], Lrelu, alpha=NEG_SLOPE)
                nc.gpsimd.dma_start(out2[m * 128 : (m + 1) * 128, :], tot[:])

            nm_sweep(2, 3, nc.gpsimd)
            nm_sweep(3, 3, nc.scalar, 0, 256)
            nm_sweep(3, 3, nc.scalar, 256, 384)
            nm_sweep(3, 3, nc.sync, 384, 512)

    nc.compile()
    return nc


def _prep_inputs(features, weights, bias, edge_out, edge_in):
    features = np.asarray(features, dtype=np.float32)
    weights = np.asarray(weights, dtype=np.float32)
    bias = np.asarray(bias, dtype=np.float32)
    ei = np.asarray(edge_in).astype(np.int64)
    eo = np.asarray(edge_out).astype(np.int64)

    # Sparse edge list -> dense [K_PAD, OUT_F] weight matrix, bias as row IN_F.
    W = np.zeros((K_PAD, OUT_F), dtype=np.float32)
    np.add.at(W, (ei, eo), weights)
    W[IN_F, :] = bias

    xT = np.zeros((K_PAD, B), dtype=np.float32)
    xT[:IN_F] = features.T
    xT[IN_F] = 1.0

    Wb = W.astype(ml_dtypes.bfloat16)
    xTb = np.ascontiguousarray(xT.astype(ml_dtypes.bfloat16))
    NA = 2 * NTILE
    maps = []
    for c in range(NCORES):
        m = {
            "xT": xTb,
            "wa": np.ascontiguousarray(Wb[:, c * SHARD : c * SHARD + NA]),
            "wb": np.ascontiguousarray(Wb[:, c * SHARD + NA : (c + 1) * SHARD]),
        }
        if FP8_CHUNKS:
            k0, k1 = FP8_CHUNKS
            r0, r1 = k0 * 128, (k1 + 1) * 128
            s = FP8_SCALE
            x8 = np.asarray(xT[r0:r1] / s, dtype=ml_dtypes.float8_e4m3)
            m["x8"] = np.ascontiguousarray(
                np.stack([x8[0:128], x8[128:256]], axis=1)
            )
            Wsc = np.asarray(
                W[r0:r1, c * SHARD : c * SHARD + 2 * NA] * s,
                dtype=ml_dtypes.float8_e4m3,
            )
            m["w8lo"] = np.ascontiguousarray(
                np.stack([Wsc[0:128, 0:NA], Wsc[128:256, 0:NA]], axis=1)
            )
            m["w8hi"] = np.ascontiguousarray(
                np.stack([Wsc[0:128, NA:], Wsc[128:256, NA:]], axis=1)
            )
            m["wt45"] = np.ascontiguousarray(
                Wb[r0:r1, c * SHARD + 2 * NA : (c + 1) * SHARD]
            )
        if E3_CHUNKS:
            blocks = [
                np.asarray(
                    W[k * 128 : (k + 1) * 128, c * SHARD : c * SHARD + NA]
                    * E3_SCALE,
                    dtype=ml_dtypes.float8_e3m4,
                )
                for k in E3_CHUNKS
            ]
            m["wa8"] = np.ascontiguousarray(np.concatenate(blocks, axis=0))
        maps.append(m)
    return maps


def _assemble(results):
    cols = []
    for c in range(NCORES):
        cols.append(results[c]["out"].astype(np.float32))
        cols.append(results[c]["out2"].astype(np.float32))
    return np.concatenate(cols, axis=1)


def kernel(features, weights, bias, edge_out, edge_in):
    from concourse import bass_utils

    in_maps = _prep_inputs(features, weights, bias, edge_out, edge_in)
    if "nc" not in _cache:
        _cache["nc"] = _build_nc()
    nc = _cache["nc"]
    res = bass_utils.run_bass_kernel_spmd(nc, in_maps, core_ids=list(range(NCORES)))
    return _assemble(res.results)
